# revision 44
# baseline (speedup 1.0000x reference)
"""ADMM-attention TRN2 kernel for nn_Attention_53034256171713.

Reference (per batch b, head h; B=8, N=1024, C=768, H=12, HD=64):
  qkv = x @ W_qkv + b_qkv -> k, v  [B,H,N,HD]
  mu = (N*C/4) / sum|k|  (per b,h);  lm = 4*mu
  6 rounds: s = soft_threshold(k - l + y/mu, lm); k2 = k - s - y/mu
            attn = softmax(k2 @ k2^T * HD^-.5); l = attn @ v; y += mu*(k-l-s)
  out = concat_h(l) @ W_proj + b_proj;  returns (out, attn)

Sharding: data-parallel over batch across the 8 NeuronCores (core i = batch
i); each core computes all 12 heads; no collectives.

Device algorithm:
  - Per-head state is d-major ([HD, N]) packed two heads per 128-partition
    tile. Tracks u = y/mu and d = k - l, removing mu from the loop (only
    lm = 4*mu survives as a per-partition scalar).
  - Softmax shift c[n] = ||k2_n||^2*SCALE + DELTA (diag of S + margin; valid
    since max_nm (S[n,m]-S[n,n])*SCALE stays < ~94 on this trajectory). The
    shift is folded into the S PSUM by one extra K=1 accumulating matmul
    with rhs = -||k2||^2; any rounding of c cancels exactly in the softmax
    ratio, so the whole c-path runs float32r. P-hat tiles [m-part, n-free]
    feed l~ = [v|1]^T @ P-hat directly (contraction over partitions): attn@v
    and row sums with no transposes; l = l~[0:64] / l~[64].
  - Numerics: the ADMM trajectory amplifies rounding ~1e4x into final attn
    logits (|S|*SCALE grows to ~730), so every matmul feeding the trajectory
    (qkv, S, l~) must be fp32-exact; float32r (1e-4) is only used where
    errors cancel (c-path) or on the output projection.
  - S = k2^T k2 runs at float32r speed but exactly: k2 = k2h + k2l (f32r
    round + f32r residual), stacked as T1=[hi;lo], T2=[lo;hi] (K=128);
    S = T1^T@T1 + T1^T@T2 reproduces all four cross terms. The lo/hi halves
    are placed by same-partition DVE copies plus partition-shifting
    SBUF->SBUF DMAs.
  - Final iteration re-exps each S tile in its n-major reading (per-partition
    bias -c[n]) with ACT accumulate for row sums, normalizes by
    reciprocal_approx_fast, and DMAs attn out; projection reloads the final
    per-pair l from a DRAM spill.
"""

import sys

if '/opt/trn_rl_repo' not in sys.path:
    sys.path.insert(0, '/opt/trn_rl_repo')

import numpy as np

B, N, C, H = 8, 1024, 768, 12
HD = C // H
SCALE = HD ** -0.5
N_ITERS = 5
NP = N // 128          # 8 n-tiles
CP = C // 128          # 6 channel tiles
NPAIR = H // 2         # 6 head-pairs
LM_NUMER = 4.0 * (N * C / 4.0)
DELTA = 50.0           # extra softmax shift margin
DEBUG = False

_RUNNER = None


def _build():
    import concourse.bacc as bacc
    import concourse.mybir as mybir
    from concourse.tile import TileContext
    from concourse import library_config

    dt = mybir.dt
    AF = mybir.ActivationFunctionType
    ALU = mybir.AluOpType
    AX = mybir.AxisListType
    f32 = dt.float32
    f32r = dt.float32r

    nc = bacc.Bacc("TRN2", target_bir_lowering=False, debug=False, num_devices=8)

    xT_d = nc.dram_tensor("xT", [C, N], f32, kind="ExternalInput")
    wqkv_d = nc.dram_tensor("wqkv", [C, 2 * C], f32, kind="ExternalInput")
    bqkv_d = nc.dram_tensor("bqkv", [2 * C], f32, kind="ExternalInput")
    wproj_d = nc.dram_tensor("wproj", [C, C], f32r, kind="ExternalInput")
    bproj_d = nc.dram_tensor("bproj", [C], f32, kind="ExternalInput")
    halfsel_d = nc.dram_tensor("halfsel", [2, 128], f32, kind="ExternalInput")
    ehalf_d = nc.dram_tensor("ehalf", [128, 2], f32, kind="ExternalInput")
    out_d = nc.dram_tensor("out", [N, C], f32, kind="ExternalOutput")
    dbg_d = nc.dram_tensor("dbg", [40, 128, N], f32, kind="ExternalOutput") if DEBUG else None
    attn_d = nc.dram_tensor("attn", [H, N, N], f32, kind="ExternalOutput")

    with TileContext(nc, num_cores=8) as tc:
        nc.gpsimd.load_library(library_config.attn)
        with tc.tile_pool(name="dram", bufs=1, space="DRAM") as dp, \
             tc.tile_pool(name="persist", bufs=1) as pp:
            ones_row = pp.tile([1, N], f32, tag="ones_row")
            nc.vector.memset(ones_row[:], 1.0)
            onesr_row = pp.tile([1, 128], f32r, tag="onesr_row")
            nc.vector.tensor_copy(onesr_row[:], ones_row[:, 0:128])
            onesr_rowN = pp.tile([1, N], f32r, tag="onesr_rowN")
            nc.vector.tensor_copy(onesr_rowN[:], ones_row[:])
            ones_col = pp.tile([128, 1], f32, tag="ones_col")
            nc.vector.memset(ones_col[:], 1.0)
            onesr_col = pp.tile([128, 1], f32r, tag="onesr_col")
            nc.vector.tensor_copy(onesr_col[:], ones_col[:])
            onesr_col2 = pp.tile([128, 2], f32r, tag="onesr_col2")
            nc.vector.tensor_copy(onesr_col2[:, 0:1], ones_col[:])
            nc.vector.tensor_copy(onesr_col2[:, 1:2], ones_col[:])
            halfsel = pp.tile([2, 128], f32, tag="halfsel")   # [j, q] = (q//64 == j)
            nc.sync.dma_start(halfsel[:], halfsel_d.ap())
            ehalf = pp.tile([128, 2], f32, tag="ehalf")       # [p, j] = (p//64 == j)
            nc.sync.dma_start(ehalf[:], ehalf_d.ap())
            ndelta = pp.tile([128, 1], f32, tag="ndelta")
            nc.vector.memset(ndelta[:], -DELTA)

            vt_sb = [pp.tile([128, NP * (HD + 1)], f32, tag=f"vt{h}", name=f"vt{h}") for h in range(H)]
            k_sb = [pp.tile([128, N], f32, tag=f"k{p}", name=f"k{p}") for p in range(NPAIR)]
            lm_t = pp.tile([128, NPAIR], f32, tag="lm")
                        # ================= phase 1: qkv =================
            with tc.tile_pool(name="qkv", bufs=1) as qp, \
                 tc.tile_pool(name="psQ", bufs=2, space="PSUM") as psQ:
                xT_t = qp.tile([128, CP * N], f32, tag="xT")
                wq_t = qp.tile([128, CP * 2 * C], f32, tag="wq")
                for ct in range(CP):
                    nc.sync.dma_start(xT_t[:, ct * N:(ct + 1) * N],
                                      xT_d.ap()[ct * 128:(ct + 1) * 128, :])
                    nc.sync.dma_start(wq_t[:, ct * 2 * C:(ct + 1) * 2 * C],
                                      wqkv_d.ap()[ct * 128:(ct + 1) * 128, :])
                bk_t = qp.tile([128, CP], f32, tag="bk")
                for m in range(CP):
                    nc.sync.dma_start(bk_t[:, m:m + 1],
                                      bqkv_d.ap()[m * 128:(m + 1) * 128].unsqueeze(1))
                bv_t = qp.tile([1, C], f32, tag="bv")
                nc.sync.dma_start(bv_t[:], bqkv_d.ap()[C:2 * C].unsqueeze(0))

                # kT (c-major): pair-tile m holds channels 128m..128m+127
                colsum = qp.tile([128, CP], f32, tag="colsum")
                for m in range(CP):
                    pk = psQ.tile([128, N], f32, tag="pk")
                    for ch in range(2):
                        for ct in range(CP):
                            nc.tensor.matmul(
                                pk[:, ch * 512:(ch + 1) * 512],
                                wq_t[:, ct * 2 * C + m * 128: ct * 2 * C + (m + 1) * 128],
                                xT_t[:, ct * N + ch * 512: ct * N + ch * 512 + 512],
                                start=(ct == 0), stop=(ct == CP - 1))
                    nc.scalar.activation(k_sb[m][:], pk[:], AF.Identity,
                                         bias=bk_t[:, m:m + 1])
                    nc.vector.tensor_reduce(colsum[:, m:m + 1], k_sb[m][:],
                                            AX.X, ALU.add, apply_absolute_value=True)

                # v (n-major) for channels C..2C; bias via K=1 ones-row matmul
                for nt in range(NP):
                    pv = psQ.tile([128, C], f32, tag="pv", bufs=1)
                    for co, cw in ((0, 512), (512, 256)):
                        for ct in range(CP):
                            nc.tensor.matmul(
                                pv[:, co:co + cw],
                                xT_t[:, ct * N + nt * 128: ct * N + (nt + 1) * 128],
                                wq_t[:, ct * 2 * C + C + co: ct * 2 * C + C + co + cw],
                                start=(ct == 0), stop=False)
                        nc.tensor.matmul(pv[:, co:co + cw], ones_row[:, 0:128],
                                         bv_t[:, co:co + cw], start=False, stop=True)
                    for h in range(H):
                        nc.vector.tensor_copy(
                            vt_sb[h][:, nt * (HD + 1): nt * (HD + 1) + HD],
                            pv[:, h * HD:(h + 1) * HD])
                for h in range(H):
                    for nt in range(NP):
                        nc.vector.tensor_copy(
                            vt_sb[h][:, nt * (HD + 1) + HD:(nt + 1) * (HD + 1)],
                            ones_col[:])

                # lm per head -> per-partition scalars [128, NPAIR]
                pmu = psQ.tile([2, NPAIR], f32, tag="pmu", bufs=1)
                nc.tensor.matmul(pmu[:], ehalf[:], colsum[:], start=True, stop=True)
                lmrow = qp.tile([2, NPAIR], f32, tag="lmrow")
                nc.vector.reciprocal(lmrow[:], pmu[:])
                nc.vector.tensor_scalar_mul(lmrow[:], lmrow[:], float(LM_NUMER))
                plm = psQ.tile([128, NPAIR], f32, tag="pmu", name="plm", bufs=1)
                nc.tensor.matmul(plm[:], halfsel[:], lmrow[:], start=True, stop=True)
                nc.vector.tensor_copy(lm_t[:], plm[:])

            if DEBUG:
                nc.sync.dma_start(dbg_d.ap()[0], k_sb[0][:])
                nc.sync.dma_start(dbg_d.ap()[1, :, 0:NP * (HD + 1)], vt_sb[0][:].bitcast(f32))
                nc.sync.dma_start(dbg_d.ap()[1, :, 600:600 + NPAIR], lm_t[:])
            # ================= phase 2: ADMM loop =================
            lfin_dram = dp.tile([C, N], f32r, tag="lfind")
            with tc.tile_pool(name="state", bufs=1) as stp, \
                 tc.tile_pool(name="work", bufs=2) as wp, \
                 tc.tile_pool(name="ppool", bufs=3) as ppool, \
                 tc.tile_pool(name="psS", bufs=2, space="PSUM") as psS, \
                 tc.tile_pool(name="psX", bufs=2, space="PSUM") as psX:
                u_sb = [stp.tile([128, N], f32, tag=f"u{p}", name=f"u{p}") for p in range(NPAIR)]
                dmi_sb = [stp.tile([128, N], f32, tag=f"d{p}", name=f"d{p}") for p in range(NPAIR)]

                def emit_pair_iter(p, it):
                    final = (it == N_ITERS)
                    lm_ap = lm_t[:, p:p + 1]
                    k = k_sb[p]
                    # ---- elementwise: s, k2 ----
                    k2 = wp.tile([128, N], f32, tag="k2", bufs=2)
                    s = None
                    if it == 0:
                        nlm = wp.tile([128, 1], f32, tag="nlm")
                        nc.vector.tensor_scalar_mul(nlm[:], lm_ap, -1.0)
                        nc.vector.tensor_scalar(k2[:], k[:], nlm[:], lm_ap,
                                                ALU.max, ALU.min)
                    else:
                        a = wp.tile([128, N], f32, tag="lp", name="a")
                        nc.vector.tensor_add(a[:], dmi_sb[p][:], u_sb[p][:])
                        t2 = wp.tile([128, N], f32, tag="t2")
                        nc.vector.tensor_scalar(t2[:], a[:], lm_ap, 0.0,
                                                ALU.add, ALU.min)
                        s = wp.tile([128, N], f32, tag="s")
                        nc.vector.scalar_tensor_tensor(s[:], a[:], lm_ap, t2[:],
                                                       ALU.subtract, ALU.max)
                        w = wp.tile([128, N], f32, tag="t2")
                        nc.vector.tensor_sub(w[:], k[:], u_sb[p][:])
                        nc.vector.tensor_sub(k2[:], w[:], s[:])

                    # ---- hi/lo split written directly from k2 ----
                    T1 = [wp.tile([128, N], f32r, tag="T1", name=f"T1{j}", bufs=2) for j in range(2)]
                    T2 = [wp.tile([128, N], f32r, tag="T2", name=f"T2{j}", bufs=2) for j in range(2)]
                    Ax = [wp.tile([65, N], f32r, tag="Ax", name=f"Ax{j}", bufs=2) for j in range(2)]
                    Bx = [wp.tile([65, N], f32r, tag="Bx", name=f"Bx{j}", bufs=2) for j in range(2)]
                    nc.vector.tensor_copy(T1[0][0:64, :], k2[0:64, :])
                    nc.vector.tensor_sub(T2[0][0:64, :], k2[0:64, :],
                                         T1[0][0:64, :].bitcast(f32))
                    nc.vector.tensor_copy(T2[1][64:128, :], k2[64:128, :])
                    nc.vector.tensor_sub(T1[1][64:128, :], k2[64:128, :],
                                         T2[1][64:128, :].bitcast(f32))
                    nc.sync.dma_start(T1[0][64:128, :], T2[0][0:64, :])
                    nc.sync.dma_start(T2[0][64:128, :], T1[0][0:64, :])
                    nc.sync.dma_start(T1[1][0:64, :], T2[1][64:128, :])
                    nc.sync.dma_start(T2[1][0:64, :], T1[1][64:128, :])
                    for j in range(2):
                        hpj = slice(64 * j, 64 * j + 64)
                        if it != N_ITERS:
                            if j == 0:
                                nc.vector.tensor_copy(Ax[0][0:64, :],
                                                      T1[0][0:64, :].bitcast(f32))
                            else:
                                nc.sync.dma_start(Ax[1][0:64, :], T2[1][64:128, :])
                            nc.sync.dma_start(Ax[j][64:65, :], onesr_rowN[:])
                    # ---- row norms rn2[n] = sum_d k2[d,n]^2 (per head) ----
                    k2sq = wp.tile([128, N], f32r, tag="at", name="k2sq")
                    nc.vector.tensor_mul(k2sq[:], k2[:], k2[:])
                    nrn = []      # [1, N] f32r rows: -rn2[n]
                    bias_nm = []  # final only: [128, NP] fp32: -(rn2*SCALE + DELTA)
                    for j in range(2):
                        hp = slice(64 * j, 64 * j + 64)
                        prn = psX.tile([1, N], f32, tag="aux")
                        for ch in range(2):
                            nc.tensor.matmul(prn[:, ch * 512:ch * 512 + 512],
                                             onesr_col[hp, :],
                                             k2sq[hp, ch * 512:ch * 512 + 512],
                                             start=True, stop=True)
                        nr = wp.tile([1, N], f32r, tag="nrn")
                        nc.vector.tensor_scalar_mul(nr[:], prn[:], -1.0)
                        nrn.append(nr)
                        if not final:
                            if j == 0:
                                nc.vector.tensor_copy(Bx[0][0:64, :],
                                                      T1[0][0:64, :].bitcast(f32))
                            else:
                                nc.sync.dma_start(Bx[1][0:64, :], T2[1][64:128, :])
                            nc.sync.dma_start(Bx[j][64:65, :], nr[:])
                        if final:
                            pnm = psX.tile([128, 2 * NP], f32, tag="aux")
                            for nt in range(NP):
                                nc.tensor.matmul(pnm[:, 2 * nt:2 * nt + 2],
                                                 k2sq[hp, nt * 128:(nt + 1) * 128],
                                                 onesr_col2[hp, :],
                                                 start=True, stop=True)
                            bnm = wp.tile([128, 2 * NP], f32, tag="bnm")
                            nc.vector.tensor_scalar(bnm[:], pnm[:], -SCALE, -DELTA,
                                                    ALU.mult, ALU.add)
                            bias_nm.append(bnm)

                    # ---- per head: S, exp, l~ ----
                    l_pair = wp.tile([128, N], f32r if final else f32, tag="lp", name="lp")
                    for j in range(2):
                        hh = 2 * p + j
                        hp = slice(64 * j, 64 * j + 64)
                        lt = psX.tile([65, N], f32, tag="aux", name="lt")
                        for nt in range(NP):
                            ps = psS.tile([128, N], f32, tag="S")
                            for ch in range(2):
                                nc.tensor.matmul(
                                    ps[:, ch * 512:ch * 512 + 512],
                                    T1[j][:, nt * 128:(nt + 1) * 128] if final
                                    else Ax[j][:, nt * 128:(nt + 1) * 128],
                                    T1[j][:, ch * 512:ch * 512 + 512] if final
                                    else Bx[j][:, ch * 512:ch * 512 + 512],
                                    start=True, stop=False)
                                nc.tensor.matmul(
                                    ps[:, ch * 512:ch * 512 + 512],
                                    T1[j][:, nt * 128:(nt + 1) * 128],
                                    T2[j][:, ch * 512:ch * 512 + 512],
                                    start=False, stop=True)
                            if final:
                                # n-major exp + rowsum, normalize, DMA attn out
                                rs = wp.tile([128, 1], f32, tag="rs")
                                at = wp.tile([128, N], f32, tag="at")
                                nc.scalar.activation(at[:], ps[:], AF.Exp,
                                                     bias=bias_nm[j][:, 2 * nt:2 * nt + 1],
                                                     scale=SCALE, accum_out=rs[:])
                                irs = wp.tile([128, 1], f32, tag="irs")
                                nc.vector.reciprocal_approx_fast(irs[:], rs[:])
                                nc.vector.tensor_scalar_mul(at[:], at[:], irs[:])
                                nc.sync.dma_start(
                                    attn_d.ap()[hh, nt * 128:(nt + 1) * 128, :], at[:])
                            if final:
                                for ch in range(2):
                                    nc.tensor.matmul(
                                        ps[:, ch * 512:ch * 512 + 512],
                                        onesr_row[:],
                                        nrn[j][:, ch * 512:ch * 512 + 512],
                                        start=False, stop=True, skip_group_check=True)
                            pt = ppool.tile([128, N], f32, tag="P")
                            nc.scalar.activation(pt[:], ps[:], AF.Exp,
                                                 bias=ndelta[:], scale=SCALE)
                            for ch in range(2):
                                nc.tensor.matmul(
                                    lt[:, ch * 512:ch * 512 + 512],
                                    vt_sb[hh][:, nt * (HD + 1):(nt + 1) * (HD + 1)],
                                    pt[:, ch * 512:ch * 512 + 512],
                                    start=(nt == 0), stop=(nt == NP - 1))
                        if DEBUG and p == 0 and j == 0:
                            dbg_lt0 = wp.tile([65, N], f32, tag="dbglt", name="dbg_lt0", bufs=1)
                            nc.vector.tensor_copy(dbg_lt0[:], lt[:])
                        # l = lt[0:64] / lt[64]
                        den_row = wp.tile([1, N], f32, tag="irsx", name="den_row", bufs=2)
                        nc.vector.tensor_copy(den_row[:], lt[64:65, :])
                        irs_row = wp.tile([1, N], f32, tag="irsx", name="irs_row", bufs=2)
                        nc.vector.reciprocal_approx_fast(irs_row[:], den_row[:])
                        irs_b = wp.tile([64, N], f32, tag="irsx", name="irs_b", bufs=2)
                        nc.gpsimd.partition_broadcast(irs_b[:], irs_row[:])
                        nc.vector.tensor_mul(l_pair[hp, :], lt[0:64, :], irs_b[:])

                    if DEBUG and p == 0:
                        base = 2 + it * 6
                        nc.sync.dma_start(dbg_d.ap()[base], k2[:])
                        nc.sync.dma_start(dbg_d.ap()[base + 1, :, :], l_pair[:].bitcast(f32) if final else l_pair[:])
                        nc.sync.dma_start(dbg_d.ap()[base + 2, 0:1, :], nrn[0][:].bitcast(f32))
                        nc.sync.dma_start(dbg_d.ap()[base + 3, 0:65, :], dbg_lt0)
                        if not final:
                            pass
                    if final:
                        nc.sync.dma_start(lfin_dram[p * 128:(p + 1) * 128, :], l_pair[:])
                    # ---- state update (skip on final) ----
                    if not final:
                        nc.vector.tensor_sub(dmi_sb[p][:], k[:], l_pair[:])
                        if it == 0:
                            nc.vector.tensor_sub(u_sb[p][:], k2[:], l_pair[:])
                        else:
                            nc.vector.tensor_add(u_sb[p][:], u_sb[p][:], dmi_sb[p][:])
                            nc.vector.tensor_sub(u_sb[p][:], u_sb[p][:], s[:])
                    if DEBUG and p == 0 and not final:
                        nc.sync.dma_start(dbg_d.ap()[2 + it * 6 + 4], u_sb[p][:])

                for it in range(N_ITERS + 1):
                    for p in range(NPAIR):
                        emit_pair_iter(p, it)

            # ================= phase 3: projection =================
            with tc.tile_pool(name="proj", bufs=1) as prp, \
                 tc.tile_pool(name="projw", bufs=3) as prw, \
                 tc.tile_pool(name="psP", bufs=2, space="PSUM") as psP:
                wp_t = prp.tile([128, CP * C], f32r, tag="wp")
                for ct in range(CP):
                    nc.sync.dma_start(wp_t[:, ct * C:(ct + 1) * C],
                                      wproj_d.ap()[ct * 128:(ct + 1) * 128, :])
                lf_t = prp.tile([128, CP * N], f32r, tag="lf")
                for ct in range(CP):
                    nc.sync.dma_start(lf_t[:, ct * N:(ct + 1) * N],
                                      lfin_dram[ct * 128:(ct + 1) * 128, :])
                bp_t = prp.tile([1, C], f32, tag="bp")
                nc.sync.dma_start(bp_t[:], bproj_d.ap().unsqueeze(0))
                for nt in range(NP):
                    po = psP.tile([128, C], f32, tag="po")
                    for co, cw in ((0, 512), (512, 256)):
                        for ct in range(CP):
                            nc.tensor.matmul(
                                po[:, co:co + cw],
                                lf_t[:, ct * N + nt * 128: ct * N + nt * 128 + 128],
                                wp_t[:, ct * C + co: ct * C + co + cw],
                                start=(ct == 0), stop=False)
                        nc.tensor.matmul(po[:, co:co + cw], ones_row[:, 0:128],
                                         bp_t[:, co:co + cw], start=False, stop=True)
                    ot = prw.tile([128, C], f32, tag="ot")
                    nc.vector.tensor_copy(ot[:], po[:])
                    nc.sync.dma_start(out_d.ap()[nt * 128:(nt + 1) * 128, :], ot[:])

    nc.finalize()
    return nc


def _get_runner():
    global _RUNNER
    if _RUNNER is None:
        _RUNNER = _build()
    return _RUNNER


def make_in_maps(inputs: dict) -> list:
    x = np.asarray(inputs["x"], dtype=np.float32)
    xT = np.ascontiguousarray(x.transpose(0, 2, 1))  # [B, C, N]
    hs = np.zeros((2, 128), np.float32); hs[0, :64] = 1.0; hs[1, 64:] = 1.0
    eh = np.zeros((128, 2), np.float32); eh[:64, 0] = 1.0; eh[64:, 1] = 1.0
    shared = {
        "halfsel": hs, "ehalf": eh,
        "wqkv": np.ascontiguousarray(inputs["W_qkv"], dtype=np.float32),
        "bqkv": np.ascontiguousarray(inputs["b_qkv"], dtype=np.float32),
        "wproj": np.ascontiguousarray(inputs["W_proj"], dtype=np.float32),
        "bproj": np.ascontiguousarray(inputs["b_proj"], dtype=np.float32),
    }
    return [dict(xT=np.ascontiguousarray(xT[b]), **shared) for b in range(B)]


def kernel(x, W_qkv, b_qkv, W_proj, b_proj):
    from concourse.bass_utils import run_bass_kernel_spmd
    nc = _get_runner()
    in_maps = make_in_maps(dict(x=x, W_qkv=W_qkv, b_qkv=b_qkv,
                                W_proj=W_proj, b_proj=b_proj))
    res = run_bass_kernel_spmd(nc, in_maps, list(range(B)))
    if DEBUG:
        try:
            np.save('/root/problem/work/hw_dbg.npy', res.results[0]["dbg"])
        except Exception:
            pass
    out = np.stack([res.results[b]["out"] for b in range(B)])
    attn = np.stack([res.results[b]["attn"] for b in range(B)])
    return out, attn


# revision 45
# speedup vs baseline: 1.0133x; 1.0133x over previous
"""ADMM-attention TRN2 kernel for nn_Attention_53034256171713.

Reference (per batch b, head h; B=8, N=1024, C=768, H=12, HD=64):
  qkv = x @ W_qkv + b_qkv -> k, v  [B,H,N,HD]
  mu = (N*C/4) / sum|k|  (per b,h);  lm = 4*mu
  6 rounds: s = soft_threshold(k - l + y/mu, lm); k2 = k - s - y/mu
            attn = softmax(k2 @ k2^T * HD^-.5); l = attn @ v; y += mu*(k-l-s)
  out = concat_h(l) @ W_proj + b_proj;  returns (out, attn)

Sharding: data-parallel over batch across the 8 NeuronCores (core i = batch
i); each core computes all 12 heads; no collectives.

Device algorithm:
  - Per-head state is d-major ([HD, N]) packed two heads per 128-partition
    tile. Tracks u = y/mu and d = k - l, removing mu from the loop (only
    lm = 4*mu survives as a per-partition scalar).
  - Softmax shift c[n] = ||k2_n||^2*SCALE + DELTA (diag of S + margin; valid
    since max_nm (S[n,m]-S[n,n])*SCALE stays < ~94 on this trajectory). The
    shift is folded into the S PSUM by one extra K=1 accumulating matmul
    with rhs = -||k2||^2; any rounding of c cancels exactly in the softmax
    ratio, so the whole c-path runs float32r. P-hat tiles [m-part, n-free]
    feed l~ = [v|1]^T @ P-hat directly (contraction over partitions): attn@v
    and row sums with no transposes; l = l~[0:64] / l~[64].
  - Numerics: the ADMM trajectory amplifies rounding ~1e4x into final attn
    logits (|S|*SCALE grows to ~730), so every matmul feeding the trajectory
    (qkv, S, l~) must be fp32-exact; float32r (1e-4) is only used where
    errors cancel (c-path) or on the output projection.
  - S = k2^T k2 runs at float32r speed but exactly: k2 = k2h + k2l (f32r
    round + f32r residual), stacked as T1=[hi;lo], T2=[lo;hi] (K=128);
    S = T1^T@T1 + T1^T@T2 reproduces all four cross terms. The lo/hi halves
    are placed by same-partition DVE copies plus partition-shifting
    SBUF->SBUF DMAs.
  - Final iteration re-exps each S tile in its n-major reading (per-partition
    bias -c[n]) with ACT accumulate for row sums, normalizes by
    reciprocal_approx_fast, and DMAs attn out; projection reloads the final
    per-pair l from a DRAM spill.
"""

import sys

if '/opt/trn_rl_repo' not in sys.path:
    sys.path.insert(0, '/opt/trn_rl_repo')

import numpy as np

B, N, C, H = 8, 1024, 768, 12
HD = C // H
SCALE = HD ** -0.5
N_ITERS = 5
NP = N // 128          # 8 n-tiles
CP = C // 128          # 6 channel tiles
NPAIR = H // 2         # 6 head-pairs
LM_NUMER = 4.0 * (N * C / 4.0)
DELTA = 50.0           # extra softmax shift margin
DEBUG = False

_RUNNER = None


def _build():
    import concourse.bacc as bacc
    import concourse.mybir as mybir
    from concourse.tile import TileContext
    from concourse import library_config

    dt = mybir.dt
    AF = mybir.ActivationFunctionType
    ALU = mybir.AluOpType
    AX = mybir.AxisListType
    f32 = dt.float32
    f32r = dt.float32r

    nc = bacc.Bacc("TRN2", target_bir_lowering=False, debug=False, num_devices=8)

    xT_d = nc.dram_tensor("xT", [C, N], f32, kind="ExternalInput")
    wqkv_d = nc.dram_tensor("wqkv", [C, 2 * C], f32, kind="ExternalInput")
    bqkv_d = nc.dram_tensor("bqkv", [2 * C], f32, kind="ExternalInput")
    wproj_d = nc.dram_tensor("wproj", [C, C], f32r, kind="ExternalInput")
    bproj_d = nc.dram_tensor("bproj", [C], f32, kind="ExternalInput")
    halfsel_d = nc.dram_tensor("halfsel", [2, 128], f32, kind="ExternalInput")
    ehalf_d = nc.dram_tensor("ehalf", [128, 2], f32, kind="ExternalInput")
    out_d = nc.dram_tensor("out", [N, C], f32, kind="ExternalOutput")
    dbg_d = nc.dram_tensor("dbg", [40, 128, N], f32, kind="ExternalOutput") if DEBUG else None
    attn_d = nc.dram_tensor("attn", [H, N, N], f32, kind="ExternalOutput")

    with TileContext(nc, num_cores=8) as tc:
        nc.gpsimd.load_library(library_config.attn)
        with tc.tile_pool(name="dram", bufs=1, space="DRAM") as dp, \
             tc.tile_pool(name="persist", bufs=1) as pp:
            ones_row = pp.tile([1, N], f32, tag="ones_row")
            nc.vector.memset(ones_row[:], 1.0)
            onesr_row = pp.tile([1, 128], f32r, tag="onesr_row")
            nc.vector.tensor_copy(onesr_row[:], ones_row[:, 0:128])
            onesr_rowN = pp.tile([1, N], f32r, tag="onesr_rowN")
            nc.vector.tensor_copy(onesr_rowN[:], ones_row[:])
            ones_col = pp.tile([128, 1], f32, tag="ones_col")
            nc.vector.memset(ones_col[:], 1.0)
            onesr_col = pp.tile([128, 1], f32r, tag="onesr_col")
            nc.vector.tensor_copy(onesr_col[:], ones_col[:])
            onesr_col2 = pp.tile([128, 2], f32r, tag="onesr_col2")
            nc.vector.tensor_copy(onesr_col2[:, 0:1], ones_col[:])
            nc.vector.tensor_copy(onesr_col2[:, 1:2], ones_col[:])
            halfsel = pp.tile([2, 128], f32, tag="halfsel")   # [j, q] = (q//64 == j)
            nc.sync.dma_start(halfsel[:], halfsel_d.ap())
            ehalf = pp.tile([128, 2], f32, tag="ehalf")       # [p, j] = (p//64 == j)
            nc.sync.dma_start(ehalf[:], ehalf_d.ap())
            ndelta = pp.tile([128, 1], f32, tag="ndelta")
            nc.vector.memset(ndelta[:], -DELTA)

            vt_sb = [pp.tile([128, NP * (HD + 1)], f32, tag=f"vt{h}", name=f"vt{h}") for h in range(H)]
            k_sb = [pp.tile([128, N], f32, tag=f"k{p}", name=f"k{p}") for p in range(NPAIR)]
            lm_t = pp.tile([128, NPAIR], f32, tag="lm")
                        # ================= phase 1: qkv =================
            with tc.tile_pool(name="qkv", bufs=1) as qp, \
                 tc.tile_pool(name="psQ", bufs=2, space="PSUM") as psQ:
                xT_t = qp.tile([128, CP * N], f32, tag="xT")
                wq_t = qp.tile([128, CP * 2 * C], f32, tag="wq")
                for ct in range(CP):
                    nc.sync.dma_start(xT_t[:, ct * N:(ct + 1) * N],
                                      xT_d.ap()[ct * 128:(ct + 1) * 128, :])
                    nc.sync.dma_start(wq_t[:, ct * 2 * C:(ct + 1) * 2 * C],
                                      wqkv_d.ap()[ct * 128:(ct + 1) * 128, :])
                bk_t = qp.tile([128, CP], f32, tag="bk")
                for m in range(CP):
                    nc.sync.dma_start(bk_t[:, m:m + 1],
                                      bqkv_d.ap()[m * 128:(m + 1) * 128].unsqueeze(1))
                bv_t = qp.tile([1, C], f32, tag="bv")
                nc.sync.dma_start(bv_t[:], bqkv_d.ap()[C:2 * C].unsqueeze(0))
                bvr_t = qp.tile([1, C], f32r, tag="bvr")
                nc.vector.tensor_copy(bvr_t[:], bv_t[:])

                # kT (c-major): pair-tile m holds channels 128m..128m+127
                colsum = qp.tile([128, CP], f32, tag="colsum")
                for m in range(CP):
                    pk = psQ.tile([128, N], f32, tag="pk")
                    for ch in range(2):
                        for ct in range(CP):
                            nc.tensor.matmul(
                                pk[:, ch * 512:(ch + 1) * 512],
                                wq_t[:, ct * 2 * C + m * 128: ct * 2 * C + (m + 1) * 128],
                                xT_t[:, ct * N + ch * 512: ct * N + ch * 512 + 512],
                                start=(ct == 0), stop=(ct == CP - 1))
                    nc.scalar.activation(k_sb[m][:], pk[:], AF.Identity,
                                         bias=bk_t[:, m:m + 1])
                    nc.vector.tensor_reduce(colsum[:, m:m + 1], k_sb[m][:],
                                            AX.X, ALU.add, apply_absolute_value=True)

                # v (n-major) for channels C..2C; bias via K=1 ones-row matmul
                for nt in range(NP):
                    pv = psQ.tile([128, C], f32, tag="pv", bufs=1)
                    for co, cw in ((0, 512), (512, 256)):
                        for ct in range(CP):
                            nc.tensor.matmul(
                                pv[:, co:co + cw],
                                xT_t[:, ct * N + nt * 128: ct * N + (nt + 1) * 128],
                                wq_t[:, ct * 2 * C + C + co: ct * 2 * C + C + co + cw],
                                start=(ct == 0), stop=False)
                        nc.tensor.matmul(pv[:, co:co + cw], onesr_rowN[:, 0:128],
                                         bvr_t[:, co:co + cw], start=False, stop=True,
                                         skip_group_check=True)
                    for h in range(H):
                        nc.vector.tensor_copy(
                            vt_sb[h][:, nt * (HD + 1): nt * (HD + 1) + HD],
                            pv[:, h * HD:(h + 1) * HD])
                for h in range(H):
                    for nt in range(NP):
                        nc.vector.tensor_copy(
                            vt_sb[h][:, nt * (HD + 1) + HD:(nt + 1) * (HD + 1)],
                            ones_col[:])

                # lm per head -> per-partition scalars [128, NPAIR]
                pmu = psQ.tile([2, NPAIR], f32, tag="pmu", bufs=1)
                nc.tensor.matmul(pmu[:], ehalf[:], colsum[:], start=True, stop=True)
                lmrow = qp.tile([2, NPAIR], f32, tag="lmrow")
                nc.vector.reciprocal(lmrow[:], pmu[:])
                nc.vector.tensor_scalar_mul(lmrow[:], lmrow[:], float(LM_NUMER))
                plm = psQ.tile([128, NPAIR], f32, tag="pmu", name="plm", bufs=1)
                nc.tensor.matmul(plm[:], halfsel[:], lmrow[:], start=True, stop=True)
                nc.vector.tensor_copy(lm_t[:], plm[:])

            if DEBUG:
                nc.sync.dma_start(dbg_d.ap()[0], k_sb[0][:])
                nc.sync.dma_start(dbg_d.ap()[1, :, 0:NP * (HD + 1)], vt_sb[0][:].bitcast(f32))
                nc.sync.dma_start(dbg_d.ap()[1, :, 600:600 + NPAIR], lm_t[:])
            # ================= phase 2: ADMM loop =================
            lfin_dram = dp.tile([C, N], f32r, tag="lfind")
            with tc.tile_pool(name="state", bufs=1) as stp, \
                 tc.tile_pool(name="work", bufs=2) as wp, \
                 tc.tile_pool(name="ppool", bufs=3) as ppool, \
                 tc.tile_pool(name="psS", bufs=2, space="PSUM") as psS, \
                 tc.tile_pool(name="psX", bufs=2, space="PSUM") as psX:
                u_sb = [stp.tile([128, N], f32, tag=f"u{p}", name=f"u{p}") for p in range(NPAIR)]
                dmi_sb = [stp.tile([128, N], f32, tag=f"d{p}", name=f"d{p}") for p in range(NPAIR)]

                def emit_pair_iter(p, it):
                    final = (it == N_ITERS)
                    lm_ap = lm_t[:, p:p + 1]
                    k = k_sb[p]
                    # ---- elementwise: s, k2 ----
                    k2 = wp.tile([128, N], f32, tag="k2", bufs=2)
                    s = None
                    if it == 0:
                        nlm = wp.tile([128, 1], f32, tag="nlm")
                        nc.vector.tensor_scalar_mul(nlm[:], lm_ap, -1.0)
                        nc.vector.tensor_scalar(k2[:], k[:], nlm[:], lm_ap,
                                                ALU.max, ALU.min)
                    else:
                        a = wp.tile([128, N], f32, tag="lp", name="a")
                        nc.vector.tensor_add(a[:], dmi_sb[p][:], u_sb[p][:])
                        t2 = wp.tile([128, N], f32, tag="t2")
                        nc.vector.tensor_scalar(t2[:], a[:], lm_ap, 0.0,
                                                ALU.add, ALU.min)
                        s = wp.tile([128, N], f32, tag="s")
                        nc.vector.scalar_tensor_tensor(s[:], a[:], lm_ap, t2[:],
                                                       ALU.subtract, ALU.max)
                        w = wp.tile([128, N], f32, tag="t2")
                        nc.vector.tensor_sub(w[:], k[:], u_sb[p][:])
                        nc.vector.tensor_sub(k2[:], w[:], s[:])

                    # ---- hi/lo split written directly from k2 ----
                    T1 = [wp.tile([128, N], f32r, tag="T1", name=f"T1{j}", bufs=2) for j in range(2)]
                    T2 = [wp.tile([128, N], f32r, tag="T2", name=f"T2{j}", bufs=2) for j in range(2)]
                    Ax = [wp.tile([65, N], f32r, tag="Ax", name=f"Ax{j}", bufs=2) for j in range(2)]
                    Bx = [wp.tile([65, N], f32r, tag="Bx", name=f"Bx{j}", bufs=2) for j in range(2)]
                    nc.vector.tensor_copy(T1[0][0:64, :], k2[0:64, :])
                    nc.vector.tensor_sub(T2[0][0:64, :], k2[0:64, :],
                                         T1[0][0:64, :].bitcast(f32))
                    nc.vector.tensor_copy(T2[1][64:128, :], k2[64:128, :])
                    nc.vector.tensor_sub(T1[1][64:128, :], k2[64:128, :],
                                         T2[1][64:128, :].bitcast(f32))
                    nc.sync.dma_start(T1[0][64:128, :], T2[0][0:64, :])
                    nc.sync.dma_start(T2[0][64:128, :], T1[0][0:64, :])
                    nc.sync.dma_start(T1[1][0:64, :], T2[1][64:128, :])
                    nc.sync.dma_start(T2[1][0:64, :], T1[1][64:128, :])
                    for j in range(2):
                        hpj = slice(64 * j, 64 * j + 64)
                        if it != N_ITERS:
                            if j == 0:
                                nc.vector.tensor_copy(Ax[0][0:64, :],
                                                      T1[0][0:64, :].bitcast(f32))
                            else:
                                nc.sync.dma_start(Ax[1][0:64, :], T2[1][64:128, :])
                            nc.sync.dma_start(Ax[j][64:65, :], onesr_rowN[:])
                    # ---- row norms rn2[n] = sum_d k2[d,n]^2 (per head) ----
                    k2sq = wp.tile([128, N], f32r, tag="at", name="k2sq")
                    nc.vector.tensor_mul(k2sq[:], k2[:], k2[:])
                    nrn = []      # [1, N] f32r rows: -rn2[n]
                    bias_nm = []  # final only: [128, NP] fp32: -(rn2*SCALE + DELTA)
                    for j in range(2):
                        hp = slice(64 * j, 64 * j + 64)
                        prn = psX.tile([1, N], f32, tag="aux")
                        for ch in range(2):
                            nc.tensor.matmul(prn[:, ch * 512:ch * 512 + 512],
                                             onesr_col[hp, :],
                                             k2sq[hp, ch * 512:ch * 512 + 512],
                                             start=True, stop=True)
                        nr = wp.tile([1, N], f32r, tag="nrn")
                        nc.vector.tensor_scalar_mul(nr[:], prn[:], -1.0)
                        nrn.append(nr)
                        if not final:
                            if j == 0:
                                nc.vector.tensor_copy(Bx[0][0:64, :],
                                                      T1[0][0:64, :].bitcast(f32))
                            else:
                                nc.sync.dma_start(Bx[1][0:64, :], T2[1][64:128, :])
                            nc.sync.dma_start(Bx[j][64:65, :], nr[:])
                        if final:
                            pnm = psX.tile([128, 2 * NP], f32, tag="aux")
                            for nt in range(NP):
                                nc.tensor.matmul(pnm[:, 2 * nt:2 * nt + 2],
                                                 k2sq[hp, nt * 128:(nt + 1) * 128],
                                                 onesr_col2[hp, :],
                                                 start=True, stop=True)
                            bnm = wp.tile([128, 2 * NP], f32, tag="bnm")
                            nc.vector.tensor_scalar(bnm[:], pnm[:], -SCALE, -DELTA,
                                                    ALU.mult, ALU.add)
                            bias_nm.append(bnm)

                    # ---- per head: S, exp, l~ ----
                    l_pair = wp.tile([128, N], f32r if final else f32, tag="lp", name="lp")
                    for j in range(2):
                        hh = 2 * p + j
                        hp = slice(64 * j, 64 * j + 64)
                        lt = psX.tile([65, N], f32, tag="aux", name="lt")
                        for nt in range(NP):
                            ps = psS.tile([128, N], f32, tag="S")
                            for ch in range(2):
                                nc.tensor.matmul(
                                    ps[:, ch * 512:ch * 512 + 512],
                                    T1[j][:, nt * 128:(nt + 1) * 128] if final
                                    else Ax[j][:, nt * 128:(nt + 1) * 128],
                                    T1[j][:, ch * 512:ch * 512 + 512] if final
                                    else Bx[j][:, ch * 512:ch * 512 + 512],
                                    start=True, stop=False)
                                nc.tensor.matmul(
                                    ps[:, ch * 512:ch * 512 + 512],
                                    T1[j][:, nt * 128:(nt + 1) * 128],
                                    T2[j][:, ch * 512:ch * 512 + 512],
                                    start=False, stop=True)
                            if final:
                                # n-major exp + rowsum, normalize, DMA attn out
                                rs = wp.tile([128, 1], f32, tag="rs")
                                at = wp.tile([128, N], f32, tag="at")
                                nc.scalar.activation(at[:], ps[:], AF.Exp,
                                                     bias=bias_nm[j][:, 2 * nt:2 * nt + 1],
                                                     scale=SCALE, accum_out=rs[:])
                                irs = wp.tile([128, 1], f32, tag="irs")
                                nc.vector.reciprocal_approx_fast(irs[:], rs[:])
                                nc.vector.tensor_scalar_mul(at[:], at[:], irs[:])
                                nc.sync.dma_start(
                                    attn_d.ap()[hh, nt * 128:(nt + 1) * 128, :], at[:])
                            if final:
                                for ch in range(2):
                                    nc.tensor.matmul(
                                        ps[:, ch * 512:ch * 512 + 512],
                                        onesr_row[:],
                                        nrn[j][:, ch * 512:ch * 512 + 512],
                                        start=False, stop=True, skip_group_check=True)
                            pt = ppool.tile([128, N], f32, tag="P")
                            nc.scalar.activation(pt[:], ps[:], AF.Exp,
                                                 bias=ndelta[:], scale=SCALE)
                            for ch in range(2):
                                nc.tensor.matmul(
                                    lt[:, ch * 512:ch * 512 + 512],
                                    vt_sb[hh][:, nt * (HD + 1):(nt + 1) * (HD + 1)],
                                    pt[:, ch * 512:ch * 512 + 512],
                                    start=(nt == 0), stop=(nt == NP - 1))
                        if DEBUG and p == 0 and j == 0:
                            dbg_lt0 = wp.tile([65, N], f32, tag="dbglt", name="dbg_lt0", bufs=1)
                            nc.vector.tensor_copy(dbg_lt0[:], lt[:])
                        # l = lt[0:64] / lt[64]
                        den_row = wp.tile([1, N], f32, tag="irsx", name="den_row", bufs=2)
                        nc.vector.tensor_copy(den_row[:], lt[64:65, :])
                        irs_row = wp.tile([1, N], f32, tag="irsx", name="irs_row", bufs=2)
                        nc.vector.reciprocal_approx_fast(irs_row[:], den_row[:])
                        irs_b = wp.tile([64, N], f32, tag="irsx", name="irs_b", bufs=2)
                        nc.gpsimd.partition_broadcast(irs_b[:], irs_row[:])
                        nc.vector.tensor_mul(l_pair[hp, :], lt[0:64, :], irs_b[:])

                    if DEBUG and p == 0:
                        base = 2 + it * 6
                        nc.sync.dma_start(dbg_d.ap()[base], k2[:])
                        nc.sync.dma_start(dbg_d.ap()[base + 1, :, :], l_pair[:].bitcast(f32) if final else l_pair[:])
                        nc.sync.dma_start(dbg_d.ap()[base + 2, 0:1, :], nrn[0][:].bitcast(f32))
                        nc.sync.dma_start(dbg_d.ap()[base + 3, 0:65, :], dbg_lt0)
                        if not final:
                            pass
                    if final:
                        nc.sync.dma_start(lfin_dram[p * 128:(p + 1) * 128, :], l_pair[:])
                    # ---- state update (skip on final) ----
                    if not final:
                        nc.vector.tensor_sub(dmi_sb[p][:], k[:], l_pair[:])
                        if it == 0:
                            nc.vector.tensor_sub(u_sb[p][:], k2[:], l_pair[:])
                        else:
                            nc.vector.tensor_add(u_sb[p][:], u_sb[p][:], dmi_sb[p][:])
                            nc.vector.tensor_sub(u_sb[p][:], u_sb[p][:], s[:])
                    if DEBUG and p == 0 and not final:
                        nc.sync.dma_start(dbg_d.ap()[2 + it * 6 + 4], u_sb[p][:])

                for it in range(N_ITERS + 1):
                    for p in range(NPAIR):
                        emit_pair_iter(p, it)

            # ================= phase 3: projection =================
            with tc.tile_pool(name="proj", bufs=1) as prp, \
                 tc.tile_pool(name="projw", bufs=3) as prw, \
                 tc.tile_pool(name="psP", bufs=2, space="PSUM") as psP:
                wp_t = prp.tile([128, CP * C], f32r, tag="wp")
                for ct in range(CP):
                    nc.sync.dma_start(wp_t[:, ct * C:(ct + 1) * C],
                                      wproj_d.ap()[ct * 128:(ct + 1) * 128, :])
                lf_t = prp.tile([128, CP * N], f32r, tag="lf")
                for ct in range(CP):
                    nc.sync.dma_start(lf_t[:, ct * N:(ct + 1) * N],
                                      lfin_dram[ct * 128:(ct + 1) * 128, :])
                bp_t = prp.tile([1, C], f32, tag="bp")
                nc.sync.dma_start(bp_t[:], bproj_d.ap().unsqueeze(0))
                bpr_t = prp.tile([1, C], f32r, tag="bpr")
                nc.vector.tensor_copy(bpr_t[:], bp_t[:])
                for nt in range(NP):
                    po = psP.tile([128, C], f32, tag="po")
                    for co, cw in ((0, 512), (512, 256)):
                        for ct in range(CP):
                            nc.tensor.matmul(
                                po[:, co:co + cw],
                                lf_t[:, ct * N + nt * 128: ct * N + nt * 128 + 128],
                                wp_t[:, ct * C + co: ct * C + co + cw],
                                start=(ct == 0), stop=False)
                        nc.tensor.matmul(po[:, co:co + cw], onesr_rowN[:, 0:128],
                                         bpr_t[:, co:co + cw], start=False, stop=True,
                                         skip_group_check=True)
                    ot = prw.tile([128, C], f32, tag="ot")
                    nc.vector.tensor_copy(ot[:], po[:])
                    nc.sync.dma_start(out_d.ap()[nt * 128:(nt + 1) * 128, :], ot[:])

    nc.finalize()
    return nc


def _get_runner():
    global _RUNNER
    if _RUNNER is None:
        _RUNNER = _build()
    return _RUNNER


def make_in_maps(inputs: dict) -> list:
    x = np.asarray(inputs["x"], dtype=np.float32)
    xT = np.ascontiguousarray(x.transpose(0, 2, 1))  # [B, C, N]
    hs = np.zeros((2, 128), np.float32); hs[0, :64] = 1.0; hs[1, 64:] = 1.0
    eh = np.zeros((128, 2), np.float32); eh[:64, 0] = 1.0; eh[64:, 1] = 1.0
    shared = {
        "halfsel": hs, "ehalf": eh,
        "wqkv": np.ascontiguousarray(inputs["W_qkv"], dtype=np.float32),
        "bqkv": np.ascontiguousarray(inputs["b_qkv"], dtype=np.float32),
        "wproj": np.ascontiguousarray(inputs["W_proj"], dtype=np.float32),
        "bproj": np.ascontiguousarray(inputs["b_proj"], dtype=np.float32),
    }
    return [dict(xT=np.ascontiguousarray(xT[b]), **shared) for b in range(B)]


def kernel(x, W_qkv, b_qkv, W_proj, b_proj):
    from concourse.bass_utils import run_bass_kernel_spmd
    nc = _get_runner()
    in_maps = make_in_maps(dict(x=x, W_qkv=W_qkv, b_qkv=b_qkv,
                                W_proj=W_proj, b_proj=b_proj))
    res = run_bass_kernel_spmd(nc, in_maps, list(range(B)))
    if DEBUG:
        try:
            np.save('/root/problem/work/hw_dbg.npy', res.results[0]["dbg"])
        except Exception:
            pass
    out = np.stack([res.results[b]["out"] for b in range(B)])
    attn = np.stack([res.results[b]["attn"] for b in range(B)])
    return out, attn
